# revision 21
# baseline (speedup 1.0000x reference)
"""Multi-head attention Bass kernel for Trainium2, sharded over 8 NeuronCores.

Problem: B=2, S=2048, D=768, H=12 heads (d_k=64). Returns (output, attention_weights).

Sharding (data + head parallel): core c handles batch b = c//4 and heads
h0 = (c%4)*3 .. h0+3 (3 heads). W_q/W_k/W_v are split column-wise, W_o row-wise
over heads. Each core computes its 3 heads' [S,S] attention weights and a partial
output projection; the host sums the 4 partial outputs per batch and re-transposes
the attention weights.

Device-side layout choice: everything is computed in "transposed" orientation
(scores^T = [k, q]) so that the second attention matmul (P @ V) needs no on-chip
transposes: lhsT = V_aug [k, d+1] (with a ones column appended to get sum(exp) for
free), rhs = E^T [k, q]. Attention weights are written to DRAM as P^T [h, k, q]
and un-transposed on the host during gather. Matmuls run in float32r (fp32 storage,
~11-bit mantissa PE rounding, full PE rate).
"""
import numpy as np

import concourse.bass as bass
import concourse.tile as tile
from concourse import bacc, mybir, bass_utils

F32 = mybir.dt.float32
F32R = mybir.dt.float32r
AF = mybir.ActivationFunctionType

B = 2
S = 2048
D = 768
H_TOT = 12
DK = 64
H = 3            # heads per core
N_CORES = 8
ST = S // 128    # 16 seq tiles
DT = D // 128    # 6 d-model tiles
QH = 1024        # q-half size
NQH = S // QH    # 2
SCALE = 1.0 / np.sqrt(DK)

_CACHED_NC = None


def build_nc():
    nc = bacc.Bacc("TRN2", target_bir_lowering=False, debug=False, num_devices=N_CORES)

    xq = nc.dram_tensor("xq", [D, S], F32R, kind="ExternalInput").ap()  # pre-transposed on host
    xk = nc.dram_tensor("xk", [D, S], F32R, kind="ExternalInput").ap()
    xv = nc.dram_tensor("xv", [D, S], F32R, kind="ExternalInput").ap()
    wq = nc.dram_tensor("wq", [D, H * DK], F32R, kind="ExternalInput").ap()
    wk = nc.dram_tensor("wk", [D, H * DK], F32R, kind="ExternalInput").ap()
    wv = nc.dram_tensor("wv", [D, 256], F32R, kind="ExternalInput").ap()  # host-padded to 256 cols
    wo = nc.dram_tensor("wo", [H * DK, D], F32R, kind="ExternalInput").ap()
    bo = nc.dram_tensor("bo", [D], F32, kind="ExternalInput").ap()

    pt = nc.dram_tensor("pt", [H, S, S], F32, kind="ExternalOutput").ap()
    yt = nc.dram_tensor("yt", [D, S], F32, kind="ExternalOutput").ap()

    with tile.TileContext(nc) as tc:
        _emit(nc, tc, xq, xk, xv, wq, wk, wv, wo, bo, pt, yt)
    nc.compile()
    return nc


def _emit(nc, tc, xq, xk, xv, wq, wk, wv, wo, bo, pt, yt):
    from contextlib import ExitStack

    ctx = ExitStack()
    singles = ctx.enter_context(tc.tile_pool(name="singles", bufs=1))
    # Q^T / K^T head-packed tiles: tile 0 = heads 0,1 (partitions 0-63 / 64-127),
    # tile 1 = head 2 (partitions 0-63).
    qkt_pool = ctx.enter_context(tc.tile_pool(name="qkt", bufs=1))
    vaug_pool = ctx.enter_context(tc.tile_pool(name="vaug", bufs=1))

    # Weights: [768, 192] -> [128, 6, 192] (partition p, d-tile t, out col n)
    wq_t = singles.tile([128, DT, H * DK], F32R)
    wk_t = singles.tile([128, DT, H * DK], F32R)
    nc.sync.dma_start(wq_t[:], wq.rearrange("(t p) n -> p t n", p=128))
    nc.sync.dma_start(wk_t[:], wk.rearrange("(t p) n -> p t n", p=128))
    # V weights padded to 256 cols (f32r needs moving dim >= 256 for full rate)
    wv_t = singles.tile([128, DT, 256], F32R)
    nc.sync.dma_start(wv_t[:], wv.rearrange("(t p) n -> p t n", p=128))
    ones_f32 = singles.tile([128, 1], F32)
    nc.vector.memset(ones_f32[:], 1.0)

    qt_tiles = [qkt_pool.tile([128, S], F32R, tag=f"qt{m}", name=f"qt{m}") for m in range(2)]
    kt_tiles = [qkt_pool.tile([128, S], F32R, tag=f"kt{m}", name=f"kt{m}") for m in range(2)]
    vaug = [vaug_pool.tile([128, H, DK + 1], F32R, tag=f"va{m}", name=f"va{m}") for m in range(ST)]

    def head_slice(tiles, h):
        t = tiles[h // 2]
        p0 = (h % 2) * DK
        return t, p0

    # ---------------- Phase A: load x^T (v first), V/Q/K projections ----------------
    with tc.tile_pool(name="xt", bufs=3) as xt_pool, \
         tc.tile_pool(name="ps1", bufs=2, space="PSUM") as ps1, \
         tc.tile_pool(name="ps2", bufs=1, space="PSUM") as ps2:

        xts = {}
        for which, xin in ((2, xv), (0, xq), (1, xk)):
            xin_r = xin.rearrange("(t p) s -> p t s", p=128)
            xt = xt_pool.tile([128, DT, S], F32R, tag="xt", name=f"xt{which}")
            xts[which] = xt
            for dt in range(DT):
                nc.sync.dma_start(xt[:, dt, :], xin_r[:, dt, :])

        # V projection -> natural layout [seq, dout], build V_aug with ones col
        xt = xts[2]
        for m in range(ST):
            pv = ps1.tile([128, 256], F32, tag="pv", name="pv")
            for dt in range(DT):
                nc.tensor.matmul(
                    pv[:],
                    xt[:, dt, m * 128 : (m + 1) * 128],
                    wv_t[:, dt, :],
                    start=(dt == 0),
                    stop=(dt == DT - 1),
                )
            va = vaug[m]
            nc.vector.tensor_copy(
                va[:, :, 0:DK],
                pv[:, 0 : H * DK].rearrange("p (h d) -> p h d", h=H),
            )
            nc.vector.tensor_copy(va[:, :, DK : DK + 1], ones_f32[:].to_broadcast((128, H, 1)))

        # Q^T / K^T projections (transposed layout [dout, seq])
        for which in (0, 1):
            xt = xts[which]
            w_t = wq_t if which == 0 else wk_t
            dst = qt_tiles if which == 0 else kt_tiles
            for m in range(2):  # head-pair tile
                cols = slice(m * 128, m * 128 + (128 if m == 0 else 64))
                npart = 128 if m == 0 else 64
                pp = ps2.tile([128, S], F32, tag="proj", name="pp")
                for dt in range(DT):
                    for j in range(S // 512):
                        nc.tensor.matmul(
                            pp[:npart, j * 512 : (j + 1) * 512],
                            w_t[:, dt, cols],
                            xt[:, dt, j * 512 : (j + 1) * 512],
                            start=(dt == 0),
                            stop=(dt == DT - 1),
                        )
                nc.vector.tensor_copy(dst[m][:npart, :], pp[:npart, :])

    # ---------------- Phase B+C: attention per (q-half, head) + output proj ----------------
    an_tiles = {}
    with tc.tile_pool(name="et", bufs=1) as et_pool, \
         tc.tile_pool(name="an", bufs=1) as an_pool, \
         tc.tile_pool(name="wos", bufs=1) as wos_pool, \
         tc.tile_pool(name="small", bufs=1) as small_pool, \
         tc.tile_pool(name="rbc", bufs=2) as rbc_pool, \
         tc.tile_pool(name="rdram", bufs=2, space="DRAM") as rdram_pool, \
         tc.tile_pool(name="ptn", bufs=1) as ptn_pool, \
         tc.tile_pool(name="ysb", bufs=1) as y_pool, \
         tc.tile_pool(name="ps_s", bufs=3, space="PSUM") as ps_s, \
         tc.tile_pool(name="ps_o", bufs=1, space="PSUM") as ps_o:

        # W_o: [192, 768] -> [64, 3, 768] (partition = within-head row, head, col)
        wo_t = wos_pool.tile([64, H, D], F32R)
        nc.sync.dma_start(wo_t[:], wo.rearrange("(h p) n -> p h n", p=DK))
        # b_o: [768] -> [128, 6]
        bo_t = wos_pool.tile([128, DT], F32)
        nc.sync.dma_start(bo_t[:], bo.rearrange("(t p) -> p t", p=128))

        for qh in range(NQH):
            for h in range(H):
                kt_t, kp0 = head_slice(kt_tiles, h)
                qt_t, qp0 = head_slice(qt_tiles, h)
                o_ps = ps_o.tile([DK + 1, QH], F32, tag="ops", name="o_ps")
                ets = []
                for ktp in range(ST // 2):
                    s_pair = []
                    for kt in (2 * ktp, 2 * ktp + 1):
                        s_ps = ps_s.tile([128, QH], F32, tag="sps", name="s_ps")
                        s_pair.append(s_ps)
                        for j in range(QH // 512):
                            nc.tensor.matmul(
                                s_ps[:, j * 512 : (j + 1) * 512],
                                kt_t[kp0 : kp0 + DK, kt * 128 : (kt + 1) * 128],
                                qt_t[qp0 : qp0 + DK,
                                     qh * QH + j * 512 : qh * QH + (j + 1) * 512],
                                start=True,
                                stop=True,
                            )
                    for i, kt in enumerate((2 * ktp, 2 * ktp + 1)):
                        # double-buffer the first few et tags to decouple heads
                        bufs = 2 if kt < 5 else 1
                        et = et_pool.tile([128, QH], F32R, tag=f"et{kt}",
                                          name=f"et{kt}", bufs=bufs)
                        ets.append(et)
                        nc.scalar.activation(et[:], s_pair[i][:], AF.Exp, scale=float(SCALE))
                    for i, kt in enumerate((2 * ktp, 2 * ktp + 1)):
                        for j in range(QH // 512):
                            nc.tensor.matmul(
                                o_ps[:, j * 512 : (j + 1) * 512],
                                vaug[kt][:, h, :],
                                ets[kt][:, j * 512 : (j + 1) * 512],
                                start=(kt == 0),
                                stop=(kt == ST - 1),
                            )

                # free o_ps quickly: copy to SBUF, then recip/normalize from there
                o_sb = small_pool.tile([DK + 1, QH], F32, tag="osb", name="o_sb")
                nc.vector.tensor_copy(o_sb[:], o_ps[:])
                nc.vector.reciprocal(o_sb[DK : DK + 1, :], o_sb[DK : DK + 1, :])
                r_d = rdram_pool.tile([1, QH], F32, tag="rd", name="r_d")
                nc.sync.dma_start(r_d[:], o_sb[DK : DK + 1, :])
                rbc = rbc_pool.tile([128, QH], F32, tag="rbc", name="rbc")
                nc.sync.dma_start(rbc[:], r_d[:].to_broadcast((128, QH)))

                an = an_pool.tile([DK, QH], F32R, tag=f"an{h}_{qh}", name=f"an{h}_{qh}")
                an_tiles[(h, qh)] = an
                nc.vector.tensor_mul(an[:], o_sb[0:DK, :], rbc[0:DK, :])

                # normalize E^T -> P^T and write out; split DVE/GPSIMD 2:1
                for kt in range(ST):
                    et = ets[kt]
                    ptt = ptn_pool.tile([128, QH], F32, tag=f"ptn{kt % 3}",
                                        name=f"ptn_{kt % 3}")
                    if kt % 3 == 2:
                        nc.gpsimd.tensor_mul(ptt[:], et[:], rbc[:])
                    else:
                        nc.vector.tensor_mul(ptt[:], et[:], rbc[:])
                    nc.sync.dma_start(
                        pt[h, kt * 128 : (kt + 1) * 128, qh * QH : (qh + 1) * QH],
                        ptt[:],
                    )

            # output projection for this q-half (shares the ps_s PSUM slots)
            for dt in range(DT):
                y_ps = ps_s.tile([128, QH], F32, tag="sps", name="y_ps")
                for j in range(QH // 512):
                    for h in range(H):
                        nc.tensor.matmul(
                            y_ps[:, j * 512 : (j + 1) * 512],
                            wo_t[:, h, dt * 128 : (dt + 1) * 128],
                            an_tiles[(h, qh)][:, j * 512 : (j + 1) * 512],
                            start=(h == 0),
                            stop=(h == H - 1),
                        )
                y_sb = y_pool.tile([128, QH], F32, tag="y", name="y_sb")
                nc.scalar.activation(
                    y_sb[:], y_ps[:], AF.Identity, bias=bo_t[:, dt : dt + 1], scale=1.0
                )
                nc.sync.dma_start(
                    yt[dt * 128 : (dt + 1) * 128, qh * QH : (qh + 1) * QH], y_sb[:]
                )

    ctx.close()


def xt_d_rhs(xt_tiles, dt, j):
    return xt_tiles[dt][:, j * 512 : (j + 1) * 512]


def _get_nc():
    global _CACHED_NC
    if _CACHED_NC is None:
        _CACHED_NC = build_nc()
    return _CACHED_NC


def _pad_cols(a, n):
    out = np.zeros((a.shape[0], n), dtype=np.float32)
    out[:, : a.shape[1]] = a
    return out


def _make_in_maps(query, key, value, W_q, W_k, W_v, W_o, b_o):
    in_maps = []
    for c in range(N_CORES):
        b = c // 4
        h0 = (c % 4) * H
        cols = slice(h0 * DK, (h0 + H) * DK)
        in_maps.append(
            {
                "xq": np.ascontiguousarray(query[b].T),
                "xk": np.ascontiguousarray(key[b].T),
                "xv": np.ascontiguousarray(value[b].T),
                "wq": np.ascontiguousarray(W_q[:, cols]),
                "wk": np.ascontiguousarray(W_k[:, cols]),
                "wv": _pad_cols(W_v[:, cols], 256),
                "wo": np.ascontiguousarray(W_o[cols, :]),
                "bo": np.ascontiguousarray(b_o) / 4.0,
            }
        )
    return in_maps


def run_traced(inputs):
    """Run with NTFF tracing to get HW exec time (test-only helper)."""
    nc = _get_nc()
    in_maps = _make_in_maps(
        inputs["query"], inputs["key"], inputs["value"],
        inputs["W_q"], inputs["W_k"], inputs["W_v"], inputs["W_o"], inputs["b_o"],
    )
    return bass_utils.run_bass_kernel_spmd(
        nc, in_maps, core_ids=list(range(N_CORES)), trace=True
    )


def kernel(query, key, value, W_q, W_k, W_v, W_o, b_o):
    query = np.asarray(query, dtype=np.float32)
    key = np.asarray(key, dtype=np.float32)
    value = np.asarray(value, dtype=np.float32)
    W_q = np.asarray(W_q, dtype=np.float32)
    W_k = np.asarray(W_k, dtype=np.float32)
    W_v = np.asarray(W_v, dtype=np.float32)
    W_o = np.asarray(W_o, dtype=np.float32)
    b_o = np.asarray(b_o, dtype=np.float32)

    nc = _get_nc()
    in_maps = _make_in_maps(query, key, value, W_q, W_k, W_v, W_o, b_o)
    res = bass_utils.run_bass_kernel_spmd(nc, in_maps, core_ids=list(range(N_CORES)))

    attn = np.empty((B, H_TOT, S, S), dtype=np.float32)
    out = np.zeros((B, S, D), dtype=np.float32)
    for c in range(N_CORES):
        b = c // 4
        h0 = (c % 4) * H
        r = res.results[c]
        ptc = r["pt"]  # [H, S(k), S(q)]
        for j in range(H):
            attn[b, h0 + j] = ptc[j].T
        out[b] += r["yt"].T  # [D, S] -> [S, D]
    return out, attn


# revision 22
# speedup vs baseline: 1.2204x; 1.2204x over previous
"""Multi-head attention Bass kernel for Trainium2, sharded over 8 NeuronCores.

Problem: B=2, S=2048, D=768, H=12 heads (d_k=64). Returns (output, attention_weights).

Sharding (data + head parallel): core c handles batch b = c//4 and heads
h0 = (c%4)*3 .. h0+3 (3 heads). W_q/W_k/W_v are split column-wise, W_o row-wise
over heads. Each core computes its 3 heads' [S,S] attention weights and a partial
output projection; the host sums the 4 partial outputs per batch and re-transposes
the attention weights.

Device-side layout: everything is computed in transposed orientation
(scores^T = [k, q]) so the second attention matmul (P @ V) needs no on-chip
transposes: lhsT = V_aug [k, d+1] (ones column appended -> sum(exp) for free),
rhs = E^T [k, q]. Attention weights are written to DRAM as P^T [h, k, q] and
un-transposed on the host during gather.

Precision: matmul inputs are bf16 (PE full rate); accumulation is fp32 in PSUM;
softmax (exp, reciprocal, normalize) is fp32; outputs are fp32.
"""
import numpy as np
import ml_dtypes

import concourse.bass as bass
import concourse.tile as tile
from concourse import bacc, mybir, bass_utils

F32 = mybir.dt.float32
BF16 = mybir.dt.bfloat16
AF = mybir.ActivationFunctionType

B = 2
S = 2048
D = 768
H_TOT = 12
DK = 64
H = 3            # heads per core
N_CORES = 8
ST = S // 128    # 16 seq tiles
DT = D // 128    # 6 d-model tiles
QH = 1024        # q-half size
NQH = S // QH    # 2
SCALE = 1.0 / np.sqrt(DK)

_CACHED_NC = None


def build_nc():
    nc = bacc.Bacc("TRN2", target_bir_lowering=False, debug=False, num_devices=N_CORES)

    xq = nc.dram_tensor("xq", [D, S], BF16, kind="ExternalInput").ap()  # x^T, host-prep
    xk = nc.dram_tensor("xk", [D, S], BF16, kind="ExternalInput").ap()
    xv = nc.dram_tensor("xv", [D, S], BF16, kind="ExternalInput").ap()
    wq = nc.dram_tensor("wq", [D, H * DK], BF16, kind="ExternalInput").ap()
    wk = nc.dram_tensor("wk", [D, H * DK], BF16, kind="ExternalInput").ap()
    wv = nc.dram_tensor("wv", [D, H * DK], BF16, kind="ExternalInput").ap()
    wo = nc.dram_tensor("wo", [H * DK, D], BF16, kind="ExternalInput").ap()
    bo = nc.dram_tensor("bo", [D], F32, kind="ExternalInput").ap()

    pt = nc.dram_tensor("pt", [H, S, S], F32, kind="ExternalOutput").ap()
    yt = nc.dram_tensor("yt", [D, S], F32, kind="ExternalOutput").ap()

    with tile.TileContext(nc) as tc:
        _emit(nc, tc, xq, xk, xv, wq, wk, wv, wo, bo, pt, yt)
    nc.compile()
    return nc


def _emit(nc, tc, xq, xk, xv, wq, wk, wv, wo, bo, pt, yt):
    from contextlib import ExitStack

    ctx = ExitStack()
    singles = ctx.enter_context(tc.tile_pool(name="singles", bufs=1))
    # Q^T / K^T head-packed tiles: tile 0 = heads 0,1 (partitions 0-63 / 64-127),
    # tile 1 = head 2 (partitions 0-63).
    qkt_pool = ctx.enter_context(tc.tile_pool(name="qkt", bufs=1))
    vaug_pool = ctx.enter_context(tc.tile_pool(name="vaug", bufs=1))

    # Weights: [768, 192] -> [128, 6, 192] (partition p, d-tile t, out col n)
    wq_t = singles.tile([128, DT, H * DK], BF16)
    wk_t = singles.tile([128, DT, H * DK], BF16)
    wv_t = singles.tile([128, DT, H * DK], BF16)
    nc.sync.dma_start(wq_t[:], wq.rearrange("(t p) n -> p t n", p=128))
    nc.sync.dma_start(wk_t[:], wk.rearrange("(t p) n -> p t n", p=128))
    nc.sync.dma_start(wv_t[:], wv.rearrange("(t p) n -> p t n", p=128))
    ones_bf = singles.tile([128, 1], BF16)
    nc.vector.memset(ones_bf[:], 1.0)

    qt_tiles = [qkt_pool.tile([128, S], BF16, tag=f"qt{m}", name=f"qt{m}") for m in range(2)]
    kt_tiles = [qkt_pool.tile([128, S], BF16, tag=f"kt{m}", name=f"kt{m}") for m in range(2)]
    vaug = [vaug_pool.tile([128, H, DK + 1], BF16, tag=f"va{m}", name=f"va{m}") for m in range(ST)]

    def head_slice(tiles, h):
        t = tiles[h // 2]
        p0 = (h % 2) * DK
        return t, p0

    # ---------------- Phase A: load x^T (v first), V/Q/K projections ----------------
    with tc.tile_pool(name="xt", bufs=3) as xt_pool, \
         tc.tile_pool(name="ps1", bufs=2, space="PSUM") as ps1, \
         tc.tile_pool(name="ps2", bufs=1, space="PSUM") as ps2:

        xts = {}
        for which, xin in ((2, xv), (0, xq), (1, xk)):
            xin_r = xin.rearrange("(t p) s -> p t s", p=128)
            xt = xt_pool.tile([128, DT, S], BF16, tag="xt", name=f"xt{which}")
            xts[which] = xt
            for dt in range(DT):
                nc.sync.dma_start(xt[:, dt, :], xin_r[:, dt, :])

        # V projection -> natural layout [seq, dout], build V_aug with ones col
        xt = xts[2]
        for m in range(ST):
            pv = ps1.tile([128, H * DK], F32, tag="pv", name="pv")
            for dt in range(DT):
                nc.tensor.matmul(
                    pv[:],
                    xt[:, dt, m * 128 : (m + 1) * 128],
                    wv_t[:, dt, :],
                    start=(dt == 0),
                    stop=(dt == DT - 1),
                )
            va = vaug[m]
            nc.vector.tensor_copy(
                va[:, :, 0:DK],
                pv[:].rearrange("p (h d) -> p h d", h=H),
            )
            nc.vector.tensor_copy(va[:, :, DK : DK + 1], ones_bf[:].to_broadcast((128, H, 1)))

        # Q^T / K^T projections (transposed layout [dout, seq])
        for which in (0, 1):
            xt = xts[which]
            w_t = wq_t if which == 0 else wk_t
            dst = qt_tiles if which == 0 else kt_tiles
            for m in range(2):  # head-pair tile
                cols = slice(m * 128, m * 128 + (128 if m == 0 else 64))
                npart = 128 if m == 0 else 64
                pp = ps2.tile([128, S], F32, tag="proj", name="pp")
                for dt in range(DT):
                    for j in range(S // 512):
                        nc.tensor.matmul(
                            pp[:npart, j * 512 : (j + 1) * 512],
                            w_t[:, dt, cols],
                            xt[:, dt, j * 512 : (j + 1) * 512],
                            start=(dt == 0),
                            stop=(dt == DT - 1),
                        )
                nc.vector.tensor_copy(dst[m][:npart, :], pp[:npart, :])

    # ---------------- Phase B+C: attention per (q-half, head) + output proj ----------------
    an_tiles = {}
    with tc.tile_pool(name="et", bufs=1) as et_pool, \
         tc.tile_pool(name="an", bufs=1) as an_pool, \
         tc.tile_pool(name="wos", bufs=1) as wos_pool, \
         tc.tile_pool(name="small", bufs=2) as small_pool, \
         tc.tile_pool(name="rbc", bufs=2) as rbc_pool, \
         tc.tile_pool(name="rdram", bufs=2, space="DRAM") as rdram_pool, \
         tc.tile_pool(name="ptn", bufs=1) as ptn_pool, \
         tc.tile_pool(name="ysb", bufs=2) as y_pool, \
         tc.tile_pool(name="ps_s", bufs=3, space="PSUM") as ps_s, \
         tc.tile_pool(name="ps_o", bufs=1, space="PSUM") as ps_o:

        # W_o: [192, 768] -> [64, 3, 768] (partition = within-head row, head, col)
        wo_t = wos_pool.tile([64, H, D], BF16)
        nc.sync.dma_start(wo_t[:], wo.rearrange("(h p) n -> p h n", p=DK))
        # b_o: [768] -> [128, 6]
        bo_t = wos_pool.tile([128, DT], F32)
        nc.sync.dma_start(bo_t[:], bo.rearrange("(t p) -> p t", p=128))

        for qh in range(NQH):
            for h in range(H):
                kt_t, kp0 = head_slice(kt_tiles, h)
                qt_t, qp0 = head_slice(qt_tiles, h)
                o_ps = ps_o.tile([DK + 1, QH], F32, tag="ops", name="o_ps")
                ets = []
                for ktp in range(ST // 2):
                    s_pair = []
                    for kt in (2 * ktp, 2 * ktp + 1):
                        s_ps = ps_s.tile([128, QH], F32, tag="sps", name="s_ps")
                        s_pair.append(s_ps)
                        for j in range(QH // 512):
                            nc.tensor.matmul(
                                s_ps[:, j * 512 : (j + 1) * 512],
                                kt_t[kp0 : kp0 + DK, kt * 128 : (kt + 1) * 128],
                                qt_t[qp0 : qp0 + DK,
                                     qh * QH + j * 512 : qh * QH + (j + 1) * 512],
                                start=True,
                                stop=True,
                            )
                    for i, kt in enumerate((2 * ktp, 2 * ktp + 1)):
                        et = et_pool.tile([128, QH], BF16, tag=f"et{kt}",
                                          name=f"et{kt}", bufs=2)
                        ets.append(et)
                        nc.scalar.activation(et[:], s_pair[i][:], AF.Exp, scale=float(SCALE))
                    for i, kt in enumerate((2 * ktp, 2 * ktp + 1)):
                        for j in range(QH // 512):
                            nc.tensor.matmul(
                                o_ps[:, j * 512 : (j + 1) * 512],
                                vaug[kt][:, h, :],
                                ets[kt][:, j * 512 : (j + 1) * 512],
                                start=(kt == 0),
                                stop=(kt == ST - 1),
                            )

                # free o_ps quickly: copy to SBUF, then recip/normalize from there
                o_sb = small_pool.tile([DK + 1, QH], F32, tag="osb", name="o_sb")
                nc.vector.tensor_copy(o_sb[:], o_ps[:])
                nc.vector.reciprocal(o_sb[DK : DK + 1, :], o_sb[DK : DK + 1, :])
                r_d = rdram_pool.tile([1, QH], F32, tag="rd", name="r_d")
                nc.sync.dma_start(r_d[:], o_sb[DK : DK + 1, :])
                rbc = rbc_pool.tile([128, QH], F32, tag="rbc", name="rbc")
                nc.sync.dma_start(rbc[:], r_d[:].to_broadcast((128, QH)))

                an = an_pool.tile([DK, QH], BF16, tag=f"an{h}_{qh}", name=f"an{h}_{qh}")
                an_tiles[(h, qh)] = an
                nc.vector.tensor_mul(an[:], o_sb[0:DK, :], rbc[0:DK, :])

                # normalize E^T -> P^T and write out; split DVE/GPSIMD 2:1
                for kt in range(ST):
                    et = ets[kt]
                    ptt = ptn_pool.tile([128, QH], F32, tag=f"ptn{kt % 3}",
                                        name=f"ptn_{kt % 3}")
                    if kt % 3 == 2:
                        nc.gpsimd.tensor_mul(ptt[:], et[:], rbc[:])
                    else:
                        nc.vector.tensor_mul(ptt[:], et[:], rbc[:])
                    nc.sync.dma_start(
                        pt[h, kt * 128 : (kt + 1) * 128, qh * QH : (qh + 1) * QH],
                        ptt[:],
                    )

            # output projection for this q-half (shares the ps_s PSUM slots)
            for dt in range(DT):
                y_ps = ps_s.tile([128, QH], F32, tag="sps", name="y_ps")
                for j in range(QH // 512):
                    for h in range(H):
                        nc.tensor.matmul(
                            y_ps[:, j * 512 : (j + 1) * 512],
                            wo_t[:, h, dt * 128 : (dt + 1) * 128],
                            an_tiles[(h, qh)][:, j * 512 : (j + 1) * 512],
                            start=(h == 0),
                            stop=(h == H - 1),
                        )
                y_sb = y_pool.tile([128, QH], F32, tag="y", name="y_sb")
                nc.scalar.activation(
                    y_sb[:], y_ps[:], AF.Identity, bias=bo_t[:, dt : dt + 1], scale=1.0
                )
                nc.sync.dma_start(
                    yt[dt * 128 : (dt + 1) * 128, qh * QH : (qh + 1) * QH], y_sb[:]
                )

    ctx.close()


def _make_in_maps(query, key, value, W_q, W_k, W_v, W_o, b_o):
    bf = ml_dtypes.bfloat16
    in_maps = []
    for c in range(N_CORES):
        b = c // 4
        h0 = (c % 4) * H
        cols = slice(h0 * DK, (h0 + H) * DK)
        in_maps.append(
            {
                "xq": np.ascontiguousarray(query[b].T).astype(bf),
                "xk": np.ascontiguousarray(key[b].T).astype(bf),
                "xv": np.ascontiguousarray(value[b].T).astype(bf),
                "wq": np.ascontiguousarray(W_q[:, cols]).astype(bf),
                "wk": np.ascontiguousarray(W_k[:, cols]).astype(bf),
                "wv": np.ascontiguousarray(W_v[:, cols]).astype(bf),
                "wo": np.ascontiguousarray(W_o[cols, :]).astype(bf),
                "bo": np.ascontiguousarray(b_o) / 4.0,
            }
        )
    return in_maps


def run_traced(inputs):
    """Run with NTFF tracing to get HW exec time (test-only helper)."""
    nc = _get_nc()
    in_maps = _make_in_maps(
        inputs["query"], inputs["key"], inputs["value"],
        inputs["W_q"], inputs["W_k"], inputs["W_v"], inputs["W_o"], inputs["b_o"],
    )
    return bass_utils.run_bass_kernel_spmd(
        nc, in_maps, core_ids=list(range(N_CORES)), trace=True
    )


def _get_nc():
    global _CACHED_NC
    if _CACHED_NC is None:
        _CACHED_NC = build_nc()
    return _CACHED_NC


def kernel(query, key, value, W_q, W_k, W_v, W_o, b_o):
    query = np.asarray(query, dtype=np.float32)
    key = np.asarray(key, dtype=np.float32)
    value = np.asarray(value, dtype=np.float32)
    W_q = np.asarray(W_q, dtype=np.float32)
    W_k = np.asarray(W_k, dtype=np.float32)
    W_v = np.asarray(W_v, dtype=np.float32)
    W_o = np.asarray(W_o, dtype=np.float32)
    b_o = np.asarray(b_o, dtype=np.float32)

    nc = _get_nc()
    in_maps = _make_in_maps(query, key, value, W_q, W_k, W_v, W_o, b_o)
    res = bass_utils.run_bass_kernel_spmd(nc, in_maps, core_ids=list(range(N_CORES)))

    attn = np.empty((B, H_TOT, S, S), dtype=np.float32)
    out = np.zeros((B, S, D), dtype=np.float32)
    for c in range(N_CORES):
        b = c // 4
        h0 = (c % 4) * H
        r = res.results[c]
        ptc = r["pt"]  # [H, S(k), S(q)]
        for j in range(H):
            attn[b, h0 + j] = ptc[j].T
        out[b] += r["yt"].T  # [D, S] -> [S, D]
    return out, attn
